# revision 1
# baseline (speedup 1.0000x reference)
"""Trainium2 Bass kernel for a 2-layer edge-weighted GraphSAGE network.

Strategy (8 NeuronCores, dst-sharded):
  * Host converts the edge list (src, dst, w) into the dense row-normalized
    adjacency operator A[d, s] = sum_e w_e / max(deg_d, 1), so each layer's
    weighted segment-mean becomes a dense matmul h_N = A @ h.
  * Node ids are PADDED per core to 1280 (10 k-blocks of 128), so global
    src k-blocks align exactly with rank shards: global block g = rank g//10,
    local block g%10.  No ragged blocks anywhere.
  * Layer 1: fp8 h DoubleRow x fp8 A^T (x64) -> hN^T, fully paced by the
    12.5 MB/core A^T DMA stream, which stays resident in SBUF for reuse by
    layer 2.
  * Layer 2 is COMMUTED with the linear: out = x@W2a + meanagg(x)@W2b
    = x@W2a + meanagg(x@W2b).  Each core computes y = x@W2b (64 feats)
    locally, exchanges the tiny fp8 y (80 KB/core) via TWO pipelined
    all-gathers (local blocks 0-5, then 6-9) so the second mesh hides
    under the 64-wide DoubleRow sweep of the first 24 pairs.  This halves
    L2 PE time and cuts collective traffic 4x vs gathering the 256-dim x.
  * A tiny warm-up collective sits FIRST in program order (a collective
    instruction placed later acts as a global DMA barrier and only
    triggers after the whole stream): it absorbs cross-core launch skew
    and leaves the CC engine hot for the real exchanges.
  * Scales folded into host-side weights so PSUM->SBUF moves are plain
    vector tensor_copy: W1b /= 64 (undoes A's x64), W2b *= 8 (y fp8
    headroom), W2a *= 512 (matches the 8*64 product); final activation
    applies 1/512 and adds b2.
  * The L1 tail (hN copy -> linear -> relu -> y proj -> transpose) is
    pipelined chunk-outer across vector/scalar/PE; the final linear sweeps
    chunk-outer so output activation/DMA overlap the remaining matmuls.
"""

import os
import sys
import types

sys.path.insert(0, "/opt/trn_rl_repo")

import numpy as np

import concourse.bacc as bacc
import concourse.tile as tile
from concourse import mybir
from concourse import bass_utils
from concourse.masks import make_identity

N_NODES = 10000
N_EDGES = 640000
D_IN, D_HID, D_OUT = 128, 256, 64
N_CORES = 8
P = 128
NB = N_NODES // N_CORES          # 1250 real dst nodes per core
NBP = 1280                       # padded local nodes (10 k-blocks)
LKB = NBP // P                   # 10 local k-blocks per core
KB = N_CORES * LKB               # 80 global k-blocks
KQ = KB // 2                     # 40 DoubleRow pairs
NPAD = KB * P                    # 10240 padded global nodes
ASCALE = 64.0                    # fp8 pre-scale on A (undone in W1b / out act)
YSCALE = 8.0                     # fp8 pre-scale on y, folded into W2b
OSCALE = ASCALE * YSCALE         # 512; folded into W2a on the host
F8 = mybir.dt.float8e4
F16 = mybir.dt.float16
F32 = mybir.dt.float32

# free-axis chunks of the local dst range (PSUM bank = 512 f32)
N_CHUNKS = [(0, 512), (512, 1024), (1024, NB)]
DST_BLOCKS = [(b * P, min((b + 1) * P, NB)) for b in range((NB + P - 1) // P)]
# dst 128-blocks belonging to each chunk (4, 4, 2)
CHUNK_BLOCKS = [[b for b, (d0, d1) in enumerate(DST_BLOCKS) if d0 >= n0 and d1 <= n1]
                for (n0, n1) in N_CHUNKS]

_compiled_nc = None
LAST_EXEC_NS = None


def _build_nc():
    nc = bacc.Bacc("TRN2", target_bir_lowering=False, debug=False,
                   num_devices=N_CORES)

    as_d = nc.dram_tensor("as8", [KQ // 2, P, 4 * NB], F8, kind="ExternalInput")
    h8_d = nc.dram_tensor("h8", [P, KB * D_IN], F8, kind="ExternalInput")
    ht_d = nc.dram_tensor("ht", [D_IN, NB], F16, kind="ExternalInput")
    w1_d = nc.dram_tensor("w1", [2 * D_IN, D_HID], F16, kind="ExternalInput")
    w2a_d = nc.dram_tensor("w2a", [P, 2 * D_OUT], F16, kind="ExternalInput")
    w2b_d = nc.dram_tensor("w2b", [P, 2 * D_OUT], F16, kind="ExternalInput")
    b1_d = nc.dram_tensor("b1c", [P, 2], F32, kind="ExternalInput")
    b2_d = nc.dram_tensor("b2c", [D_OUT, 1], F32, kind="ExternalInput")
    out_d = nc.dram_tensor("outT", [D_OUT, NB], F32, kind="ExternalOutput")

    with tile.TileContext(nc) as tc:
        with (
            tc.tile_pool(name="const", bufs=1) as cpool,
            tc.tile_pool(name="acache", bufs=1) as acpool,
            tc.tile_pool(name="work", bufs=1) as wpool,
            tc.tile_pool(name="dram", bufs=1, space="DRAM") as dpool,
        ):
            # ---- warm-up collective FIRST in program order: the collective
            # instruction acts as a global DMA barrier, so anywhere later its
            # trigger waits for the whole A^T stream.  Issued first, it
            # triggers immediately, wakes the CC engine, absorbs cross-core
            # launch skew, and leaves the CC hot for the real exchange.
            warm_sb = cpool.tile([1, 16], F16)
            nc.gpsimd.memset(warm_sb[:], 0.0)
            warm_in = dpool.tile([1, 16], F16)
            warm_out = dpool.tile([N_CORES, 16], F16, addr_space="Shared")
            nc.gpsimd.dma_start(out=warm_in[:], in_=warm_sb[:])
            nc.gpsimd.collective_compute(
                "AllGather", mybir.AluOpType.bypass,
                replica_groups=[list(range(N_CORES))],
                ins=[warm_in.opt()], outs=[warm_out.opt()])

            h8 = cpool.tile([P, KB * D_IN], F8)
            HH = KB * D_IN // 2
            # A^T stream packed 2 DR pairs per tile (5000 B descriptors);
            # j ascending = consumption order, spread over all 3 DMA queues.
            # Tile 0 is split in pair-halves so the sweep starts early.
            acq2 = [acpool.tile([P, 4 * NB], F8, name=f"acq2_{j}")
                    for j in range(KQ // 2)]
            H0 = 4 * D_IN
            nc.scalar.dma_start(out=h8[:, :H0], in_=h8_d[:, :H0])
            nc.sync.dma_start(out=acq2[0][:, :2 * NB], in_=as_d[0][:, :2 * NB])
            nc.gpsimd.dma_start(out=acq2[0][:, 2 * NB:], in_=as_d[0][:, 2 * NB:])
            nc.scalar.dma_start(out=h8[:, H0:HH], in_=h8_d[:, H0:HH])
            # small constants at the sync-queue head: they land early and
            # stay off the stream-drain tail (the drain gates the CC wake)
            hts = cpool.tile([P, NB], F16)
            nc.sync.dma_start(out=hts[:], in_=ht_d[:])
            w1s = cpool.tile([P, 2 * D_HID], F16)
            for k in range(2):
                nc.sync.dma_start(out=w1s[:, k * D_HID:(k + 1) * D_HID],
                                  in_=w1_d[k * P:(k + 1) * P, :])
            w2as = cpool.tile([P, 2 * D_OUT], F16)
            nc.sync.dma_start(out=w2as[:], in_=w2a_d[:])
            w2bs = cpool.tile([P, 2 * D_OUT], F16)
            nc.sync.dma_start(out=w2bs[:], in_=w2b_d[:])
            b1s = cpool.tile([P, 2], F32)
            nc.sync.dma_start(out=b1s[:], in_=b1_d[:])
            b2s = cpool.tile([D_OUT, 1], F32)
            nc.sync.dma_start(out=b2s[:], in_=b2_d[:])
            ident = cpool.tile([P, P], F16)
            make_identity(nc, ident[:])

            stream_eng = [nc.sync, nc.scalar, nc.gpsimd]
            for j in range(1, KQ // 2):
                if j == 4:
                    nc.scalar.dma_start(out=h8[:, HH:], in_=h8_d[:, HH:])
                eng = nc.sync if j == KQ // 2 - 1 else stream_eng[j % 3]
                eng.dma_start(out=acq2[j][:], in_=as_d[j])

            hNT = wpool.tile([P, NB], F16)
            xT = [wpool.tile([P, NB], F16, name=f"xT{m}") for m in range(2)]
            yT16 = wpool.tile([D_OUT, NB], F16)
            y8 = wpool.tile([P, LKB * D_OUT], F8)
            yq = wpool.tile([P, KB * D_OUT], F8)
            outsb = wpool.tile([D_OUT, NB], F32)

            def h_pair(q):
                return h8[:, (2 * q) * D_IN:(2 * q + 2) * D_IN] \
                    .rearrange("p (two f) -> p two f", two=2)

            def a_pair(q, n0, n1):
                half = acq2[q // 2][:, (q % 2) * 2 * NB:(q % 2 + 1) * 2 * NB]
                return half.rearrange("p (two d) -> p two d",
                                      two=2)[:, :, n0:n1]

            # ---- layer 1 aggregation: hN^T = (1/64) sum_q h_q^T . A_q ------
            with tc.tile_pool(name="ps1", bufs=1, space="PSUM") as ps1:
                hN_ps = ps1.tile([P, NB], F32, space="PSUM")
                for q in range(KQ):
                    for (n0, n1) in N_CHUNKS:
                        nc.tensor.matmul(out=hN_ps[:, n0:n1],
                                         lhsT=h_pair(q),
                                         rhs=a_pair(q, n0, n1),
                                         perf_mode=mybir.MatmulPerfMode.DoubleRow,
                                         start=(q == 0), stop=(q == KQ - 1))
                # hNT = 64*hN in fp16; the /64 is folded into W1b on the host
                nc.vector.tensor_copy(out=hNT[:], in_=hN_ps[:])

            # ---- L1 tail, chunk-pipelined across PE / scalar / vector ------
            # x^T = relu(W1^T.[h;hN]^T + b1); y^T = 8*(W2b^T x^T); transpose
            nc.vector.memset(y8[:, (LKB - 1) * D_OUT:], 0.0)
            cat1 = [hts, hNT]
            with (
                tc.tile_pool(name="ps2", bufs=2, space="PSUM") as ps2,
                tc.tile_pool(name="ps3", bufs=2, space="PSUM") as ps3,
                tc.tile_pool(name="ps4", bufs=2, space="PSUM") as ps4,
            ):
                for ci, (n0, n1) in enumerate(N_CHUNKS):
                    cw = n1 - n0
                    for m in range(2):
                        x_ps = ps2.tile([P, 512], F32, space="PSUM",
                                        name="x_ps")
                        for k in range(2):
                            nc.tensor.matmul(
                                out=x_ps[:, :cw],
                                lhsT=w1s[:, k * D_HID + m * P: k * D_HID + (m + 1) * P],
                                rhs=cat1[k][:, n0:n1],
                                start=(k == 0), stop=(k == 1))
                        nc.scalar.activation(
                            out=xT[m][:, n0:n1], in_=x_ps[:, :cw],
                            func=mybir.ActivationFunctionType.Relu,
                            bias=b1s[:, m:m + 1])
                    yp_ps = ps3.tile([D_OUT, 512], F32, space="PSUM",
                                     name="yp_ps")
                    for k in range(2):
                        nc.tensor.matmul(
                            out=yp_ps[:, :cw],
                            lhsT=w2bs[:, k * D_OUT:(k + 1) * D_OUT],
                            rhs=xT[k][:, n0:n1],
                            start=(k == 0), stop=(k == 1))
                    nc.vector.tensor_copy(out=yT16[:, n0:n1],
                                          in_=yp_ps[:, :cw])
                    for b in CHUNK_BLOCKS[ci]:
                        d0, d1 = DST_BLOCKS[b]
                        bw = d1 - d0
                        tps = ps4.tile([P, D_OUT], F16, space="PSUM",
                                       name="tps")
                        nc.tensor.transpose(out=tps[:bw, :],
                                            in_=yT16[:, d0:d1],
                                            identity=ident[:D_OUT, :D_OUT])
                        nc.vector.tensor_copy(
                            out=y8[:bw, b * D_OUT:(b + 1) * D_OUT],
                            in_=tps[:bw, :])

            # ---- exchange y8 in two collectives so the second mesh hides
            # under the sweep of the first's pairs.  Part a = local blocks
            # 0-3 (pairs 5r, 5r+1 per rank), part b = blocks 4-9.
            ABW = 6 * D_OUT          # part-a bytes per row
            BBW = 4 * D_OUT
            ag_in_a = dpool.tile([P, ABW], F8)
            ag_in_b = dpool.tile([P, BBW], F8)
            ag_out_a = dpool.tile([N_CORES * P, ABW], F8, addr_space="Shared")
            ag_out_b = dpool.tile([N_CORES * P, BBW], F8, addr_space="Shared")
            nc.scalar.dma_start(out=ag_in_a[:], in_=y8[:, :ABW])
            nc.sync.dma_start(out=ag_in_b[:], in_=y8[:, ABW:])
            for ag_in, ag_out in [(ag_in_a, ag_out_a), (ag_in_b, ag_out_b)]:
                nc.gpsimd.collective_compute(
                    "AllGather", mybir.AluOpType.bypass,
                    replica_groups=[list(range(N_CORES))],
                    ins=[ag_in.opt()], outs=[ag_out.opt()])
            # yq[p, k*64+f] = y_all[128k+p, f]; k = 10r + b, so per (p, r) the
            # source bytes are one contiguous run of ag_out_{a,b} row 128r+p.
            RW = LKB * D_OUT
            yqv = yq[:].rearrange("p (r f) -> p r f", r=N_CORES)
            for (r0, r1), eng in zip([(0, 2), (2, 4), (4, 6), (6, 8)],
                                     [nc.sync, nc.scalar, nc.gpsimd, nc.sync]):
                eng.dma_start(
                    out=yqv[:, r0:r1, :ABW],
                    in_=ag_out_a[r0 * P:r1 * P]
                        .rearrange("(r p) f -> p r f", p=P))
            for (r0, r1), eng in zip([(0, 2), (2, 4), (4, 6), (6, 8)],
                                     [nc.scalar, nc.gpsimd, nc.sync, nc.scalar]):
                eng.dma_start(
                    out=yqv[:, r0:r1, ABW:],
                    in_=ag_out_b[r0 * P:r1 * P]
                        .rearrange("(r p) f -> p r f", p=P))

            def yq_pair(q):
                return yq[:, (2 * q) * D_OUT:(2 * q + 2) * D_OUT] \
                    .rearrange("p (two f) -> p two f", two=2)

            QA = [5 * r + i for r in range(N_CORES) for i in range(3)]
            QB = [5 * r + i for r in range(N_CORES) for i in range(3, 5)]

            # ---- layer 2: out^T = (1/512)(W2a'^T x^T + sum_q y_q^T A_q) -----
            # W2a + part-a pairs for all chunks first (overlap exchange b),
            # then part-b pairs chunk-outer so bias/copy + store overlap.
            # o_ps as one PSUM tile per chunk: tile-granular dependency
            # tracking otherwise false-serializes the next chunk's matmuls
            # behind the previous chunk's output activation (2x822ns gaps).
            with tc.tile_pool(name="ps5", bufs=1, space="PSUM") as ps5:
                o_ps = [ps5.tile([D_OUT, 512], F32, space="PSUM",
                                 name=f"o_ps{ci}")
                        for ci in range(len(N_CHUNKS))]
                for ci, (n0, n1) in enumerate(N_CHUNKS):
                    cw = n1 - n0
                    for k in range(2):
                        nc.tensor.matmul(
                            out=o_ps[ci][:, :cw],
                            lhsT=w2as[:, k * D_OUT:(k + 1) * D_OUT],
                            rhs=xT[k][:, n0:n1],
                            start=(k == 0), stop=False)
                for ci, (n0, n1) in enumerate(N_CHUNKS):
                    for q in QA:
                        nc.tensor.matmul(
                            out=o_ps[ci][:, :n1 - n0],
                            lhsT=yq_pair(q),
                            rhs=a_pair(q, n0, n1),
                            perf_mode=mybir.MatmulPerfMode.DoubleRow,
                            start=False, stop=False)
                for ci, (n0, n1) in enumerate(N_CHUNKS):
                    cw = n1 - n0
                    for qi, q in enumerate(QB):
                        nc.tensor.matmul(
                            out=o_ps[ci][:, :cw],
                            lhsT=yq_pair(q),
                            rhs=a_pair(q, n0, n1),
                            perf_mode=mybir.MatmulPerfMode.DoubleRow,
                            start=False, stop=(qi == len(QB) - 1))
                    nc.scalar.activation(
                        out=outsb[:, n0:n1], in_=o_ps[ci][:, :cw],
                        func=mybir.ActivationFunctionType.Identity,
                        scale=1.0 / OSCALE, bias=b2s[:, 0:1])
                    nc.sync.dma_start(out=out_d[:, n0:n1],
                                      in_=outsb[:, n0:n1])

    nc.compile()
    return nc


def _get_nc():
    global _compiled_nc
    if _compiled_nc is None:
        _compiled_nc = _build_nc()
    return _compiled_nc


def _enable_profile_hook():
    """Register the NTFF profiling hook that trn_boot skips when the image's
    antenv lacks axon_hooks (profiling only; used when GNN_PROFILE=1)."""
    try:
        import antenv
        if "antenv.axon_hooks" not in sys.modules:
            mod = types.ModuleType("antenv.axon_hooks")
            _h = [None]
            mod.set_axon_ntff_profile_hook = lambda hook: _h.__setitem__(0, hook)
            mod.get_axon_ntff_profile_hook = lambda: _h[0]
            sys.modules["antenv.axon_hooks"] = mod
            antenv.axon_hooks = mod
        from trn_agent_boot.trn_boot import _ntff_profile_via_ctypes
        hook = _ntff_profile_via_ctypes("/opt/axon/libaxon_pjrt.so")
        if hook is not None:
            sys.modules["antenv.axon_hooks"].set_axon_ntff_profile_hook(hook)
            return True
    except Exception:
        pass
    return False


def _host_prep(h, w, src, dst, W1, b1, W2, b2):
    import ml_dtypes
    import scipy.sparse as sp
    deg = np.bincount(dst, minlength=N_NODES).astype(np.float32)
    w_norm = (w[:, 0] * (ASCALE / np.maximum(deg, 1.0)[dst])).astype(np.float32)
    # pad node ids per-core to 1280 so global k-blocks align with ranks
    src_pad = (NBP * (src // NB) + src % NB).astype(np.int64)
    # AT[s_pad, d] = sum of scaled w_norm over edges (s -> d): 64*A^T
    AT = sp.coo_matrix((w_norm, (src_pad, dst)),
                       shape=(NPAD, N_NODES)).toarray()
    AT8 = AT.astype(ml_dtypes.float8_e4m3)
    hp_f = np.zeros((NPAD, D_IN), dtype=np.float32)
    for c in range(N_CORES):
        hp_f[c * NBP:c * NBP + NB] = h[c * NB:(c + 1) * NB]
    hp = hp_f.astype(ml_dtypes.float8_e4m3)
    # h8[p, k*128+f] = h[pad node 128k+p, f]
    h8 = np.ascontiguousarray(
        hp.reshape(KB, P, D_IN).transpose(1, 0, 2).reshape(P, KB * D_IN))

    # W1 with the hN half pre-divided by ASCALE (hNT arrives as 64*hN)
    w1c = W1.astype(np.float16)
    w1c[D_IN:] = (W1[D_IN:] / ASCALE).astype(np.float16)
    w2ac = np.zeros((P, 2 * D_OUT), dtype=np.float16)
    w2bc = np.zeros((P, 2 * D_OUT), dtype=np.float16)
    for k in range(2):
        w2ac[:, k * D_OUT:(k + 1) * D_OUT] = (
            W2[k * P:(k + 1) * P, :] * OSCALE).astype(np.float16)
        w2bc[:, k * D_OUT:(k + 1) * D_OUT] = (
            W2[2 * P + k * P:2 * P + (k + 1) * P, :] * YSCALE).astype(np.float16)
    b1c = np.ascontiguousarray(b1.reshape(2, P).T)
    b2c = b2.reshape(D_OUT, 1)

    in_maps = []
    for c in range(N_CORES):
        sl = slice(c * NB, (c + 1) * NB)
        ATc = AT8[:, sl]
        # as8[t, p, (q2*2+j)*NB+d] = ATc[(4t+2*q2+j)*128+p, d]: two DR
        # pairs per tile, each pair-interleaved
        as8 = np.ascontiguousarray(
            ATc.reshape(KQ // 2, 2, 2, P, NB).transpose(0, 3, 1, 2, 4)
            .reshape(KQ // 2, P, 4 * NB))
        in_maps.append({
            "as8": as8,
            "h8": h8,
            "ht": np.ascontiguousarray(h[sl].T.astype(np.float16)),
            "w1": w1c,
            "w2a": w2ac,
            "w2b": w2bc,
            "b1c": b1c,
            "b2c": b2c,
        })
    return in_maps


def kernel(h, w, src, dst, W1, b1, W2, b2):
    global LAST_EXEC_NS
    h = np.asarray(h, dtype=np.float32)
    w = np.asarray(w, dtype=np.float32)
    src = np.asarray(src)
    dst = np.asarray(dst)
    W1 = np.asarray(W1, dtype=np.float32)
    b1 = np.asarray(b1, dtype=np.float32)
    W2 = np.asarray(W2, dtype=np.float32)
    b2 = np.asarray(b2, dtype=np.float32)

    in_maps = _host_prep(h, w, src, dst, W1, b1, W2, b2)
    nc = _get_nc()
    trace = os.environ.get("GNN_PROFILE") == "1" and _enable_profile_hook()
    res = bass_utils.run_bass_kernel_spmd(
        nc, in_maps, core_ids=list(range(N_CORES)), trace=trace)
    LAST_EXEC_NS = res.exec_time_ns

    out = np.concatenate(
        [res.results[c]["outT"].T for c in range(N_CORES)], axis=0)
    return out.astype(np.float32)

